# revision 23
# baseline (speedup 1.0000x reference)
"""Trainium2 Bass kernel for BaselineFeedforwardNetwork forward_trajectory.

Math (per path, T=60 sequential steps with scalar delta feedback):
    x_t = [f_t (5), d_{t-1}]                       (6,)
    h1  = relu(x_t @ W1 + b1)                      (64,)
    h2  = relu(h1 @ W2 + b2)                       (64,)
    d_t = h2 @ W3 + b3                             scalar
Output: deltas (N, T).

Kernel structure (per core, B = N/8 = 16384 paths, data-parallel over 8 cores):
  * Feature-major activations: h1/h2 stored [hidden, path]; the scalar
    feedback d never materializes between steps -- it is folded into the
    next step's first layer via the rank-1 factor W13 = W3 (outer) w1d:
        h1pre_{t+1} = W1f.T @ fT_{t+1} + W13.T @ h2T_t + (b1 + b3*w1d)
  * Two batch groups stacked on 128 partitions (block-diagonal weights) so
    every big matmul/relu uses the full 128-lane width. Per step per
    superchunk (4096 paths = 2048 stacked cols): M1 (W2 layer, 2048 cols),
    M2 = W13-feedback + W1f-features accumulated (2 x 2048 cols). That is
    the PE streaming floor: h1pre and h2pre each saturate the PE's
    128-outputs-per-column bandwidth.
  * d_t extraction costs ~0 streaming: 32 tiny 1-column matmuls per step
    use h2 itself as the stationary weight ([64, 128] slice per 128-path
    window) and W3 as the 1-column rhs, so d lands PARTITION-major
    ([path, step] exactly as the output wants) in 4 persistent PSUM banks
    (2 groups x 2 half-T banks, col = window*30 + t%30). Two drains per
    superchunk (bias b3 fused) -> SBUF, then one 3D-AP DMA per group
    writes deltas directly. No band matmul, no dstage, no output-side
    transposes.
  * h2 is bf16 (h1 stays f32r): enables the 1-col tiny matmuls (f32r
    requires even column counts) and 2x DVE throughput on the h2 relu;
    weights touching h2 (wm2h) are bf16 to match. End-to-end error vs the
    fp32 reference stays ~1e-3 (the recurrence is contractive).
  * Features are transposed on device (PE transpose via identity) into a
    DRAM staging buffer, then streamed as [10, paths] KT-step windows with
    one 3D-access-pattern DMA per window. The next superchunk's prepass is
    interleaved into the T-loop so DMA hides under compute.
  * PSUM budget: 4 io banks (M1/M2 chunk pipeline) + 4 d-accumulator banks.
"""

import os

import numpy as np

N, T, FEAT, H = 131072, 60, 5, 64
NCORES = 8
B = N // NCORES            # 16384 paths per core
SC = int(os.environ.get("K_SC", "4096"))   # paths per superchunk
NSC = B // SC              # superchunks
G = SC // 2                # paths per group (2 groups per superchunk)
CH = 512                   # matmul rhs chunk (fp32 PSUM bank limit)
NCH = G // CH              # chunks per group
IOBUFS = int(os.environ.get("K_IOBUFS", "3"))
KT_ENV = int(os.environ.get("K_KT", "3"))
H2BF = os.environ.get("K_H2DT", "bf16") == "bf16"  # h2/wm2h/w3 in bf16
FBF = os.environ.get("K_FDT", "bf16") == "bf16"    # features/fstage/wm2f in bf16
LANES = int(os.environ.get("K_LANES", "1"))        # interleaved superchunk T-loops
NW16 = G // 128            # 128-path windows per group (16)
TQ = 15                    # steps per d-accumulator drain (quarter-T)
NQ4 = T // TQ              # drains per superchunk per group (4)

_BUILD_CACHE = {}


def _build_nc():
    import concourse.bass as bass  # noqa: F401
    import concourse.mybir as mybir
    import concourse.tile as tile
    from concourse import bacc

    f32 = mybir.dt.float32
    f32r = mybir.dt.float32r
    bf16 = mybir.dt.bfloat16
    h2dt = bf16 if H2BF else f32r
    fdt = bf16 if FBF else f32r
    Relu = mybir.ActivationFunctionType.Relu
    add_op = mybir.AluOpType.add
    max_op = mybir.AluOpType.max

    nc = bacc.Bacc("TRN2", target_bir_lowering=False, debug=False)

    feats = nc.dram_tensor("features", [B, T * FEAT], f32, kind="ExternalInput")
    wm1_d = nc.dram_tensor("wm1", [128, 128], f32r, kind="ExternalInput")
    wm2h_d = nc.dram_tensor("wm2h", [128, 128], h2dt, kind="ExternalInput")
    wm2f_d = nc.dram_tensor("wm2f", [2 * FEAT, 128], fdt, kind="ExternalInput")
    w3_d = nc.dram_tensor("w3col", [128, 2], h2dt, kind="ExternalInput")
    ident_d = nc.dram_tensor("ident", [128, 128], f32, kind="ExternalInput")
    bias_h2_d = nc.dram_tensor("bias_h2", [128, 1], f32, kind="ExternalInput")
    bias_h1_d = nc.dram_tensor("bias_h1", [128, 1], f32, kind="ExternalInput")
    bias_h1f_d = nc.dram_tensor("bias_h1f", [128, 1], f32, kind="ExternalInput")
    bias_d_d = nc.dram_tensor("bias_d", [128, 1], f32, kind="ExternalInput")
    out_d = nc.dram_tensor("deltas", [B, T], f32, kind="ExternalOutput")

    # d-accumulator width: 16 windows x 15 steps; f32r tiny matmuls need a
    # 2-col write (trample-then-fix), bf16 writes 1 col.
    DOUTW = NW16 * TQ + (2 if not H2BF else 0)

    with tile.TileContext(nc) as tc:
        with (
            tc.tile_pool(name="constp", bufs=1) as constp,
            tc.tile_pool(name="iop", bufs=3) as iop,
            tc.tile_pool(name="statep", bufs=2) as statep,
            tc.tile_pool(name="pspool", bufs=IOBUFS, space="PSUM") as pspool,
            tc.tile_pool(name="doutp", bufs=2, space="PSUM") as doutp,
            tc.tile_pool(name="dramp", bufs=1, space="DRAM") as dramp,
        ):
            wm1 = constp.tile_from(wm1_d[:, :], name="wm1_sb")
            wm2h = constp.tile_from(wm2h_d[:, :], name="wm2h_sb")
            wm2f = constp.tile_from(wm2f_d[:, :], name="wm2f_sb")
            w3sb = constp.tile_from(w3_d[:, :], name="w3_sb")
            ident = constp.tile_from(ident_d[:, :], name="ident_sb")
            bias_h2 = constp.tile_from(bias_h2_d[:, :], name="bias_h2_sb")
            bias_h1 = constp.tile_from(bias_h1_d[:, :], name="bias_h1_sb")
            bias_h1f = constp.tile_from(bias_h1f_d[:, :], name="bias_h1f_sb")
            bias_d = constp.tile_from(bias_d_d[:, :], name="bias_d_sb")

            fstage = dramp.tile([T * FEAT, B], fdt, name="fstage")

            def relu_bias(engine_is_act, dst, src, bias_ap):
                if engine_is_act:
                    nc.scalar.activation(dst, src, Relu, bias=bias_ap)
                else:
                    nc.vector.tensor_scalar(dst, src, bias_ap, 0.0,
                                            add_op, max_op)

            KT = KT_ENV  # steps per batched fT window DMA
            assert T % KT == 0
            NW = SC // 512  # 512-path windows per superchunk (prepass units)

            def prepass_load(sc, w):
                """DMA 512 paths' raw features into SBUF; return j-slices."""
                base = sc * SC
                p0 = base + w * 512
                ft = iop.tile([128, 4 * T * FEAT], f32, tag="Ftile", bufs=8,
                              name="Ftile")
                src3 = feats[p0:p0 + 512, :].rearrange("(j l) c -> l j c", l=128)
                dst3 = ft.rearrange("l (j c) -> l j c", j=4)
                nc.sync.dma_start(dst3, src3)
                return ft

            def prepass_k(sc, w, ft, k):
                """Transpose rows [100k, 100k+100) of window w into fstage."""
                base = sc * SC
                p0 = base + w * 512
                ps_tr = pspool.tile([128, 2 * CH], f32, tag="io", name="ps_tr")
                for j in range(4):
                    nc.tensor.transpose(
                        ps_tr[0:100, 128 * j:128 * (j + 1)],
                        ft[:, T * FEAT * j + 100 * k:T * FEAT * j + 100 * (k + 1)],
                        ident,
                    )
                stg = iop.tile([128, 512], fdt, tag="stg", name="stg")
                nc.scalar.copy(stg[0:100, :], ps_tr[0:100, 0:512])
                nc.sync.dma_start(
                    fstage[100 * k:100 * (k + 1), p0:p0 + 512], stg[0:100, :]
                )

            def prepass_span(sc, w0, w1):
                """Stage 512-path windows [w0, w1) of sc into fstage."""
                fts = {w: prepass_load(sc, w) for w in range(w0, w1)}
                for k in range(3):
                    for w in range(w0, w1):
                        prepass_k(sc, w, fts[w], k)

            def prepass_initial(sc):
                """First superchunk: all window DMAs up front, transposes
                k-major across ALL windows so the T-loop (which consumes
                fstage rows in step order) can start after ~1/3 of the work.
                """
                fts = {w: prepass_load(sc, w) for w in range(NW)}
                for k in range(3):
                    for w in range(NW):
                        prepass_k(sc, w, fts[w], k)

            class Lane:
                pass

            def load_fwin(st, w):
                """Load fT for steps [KT*w, KT*w + KT): rows 0-4 group A, 5-9 B."""
                t0 = KT * w
                fTbig = iop.tile([2 * FEAT, KT * G], fdt, tag="fTbig", bufs=6,
                                 name="fTbig")
                for half, col in ((0, st.colA), (1, st.colB)):
                    src = fstage[FEAT * t0:FEAT * (t0 + KT), col]
                    src3 = src.rearrange("(k c) n -> c k n", c=FEAT)
                    dst3 = fTbig[FEAT * half:FEAT * (half + 1), :].rearrange(
                        "c (k n) -> c k n", n=G)
                    nc.sync.dma_start(dst3, src3)
                st.fT[w] = fTbig
                st.fT.pop(w - 2, None)

            def lane_init(sc):
                st = Lane()
                st.sc = sc
                base = sc * SC
                st.colA = slice(base, base + G)
                st.colB = slice(base + G, base + SC)
                st.fT = {}
                load_fwin(st, 0)
                load_fwin(st, 1)  # prefetch one window ahead
                st.h1 = statep.tile([128, G], f32r, tag="h1", bufs=6, name="h1")
                for hf in range(2):
                    h0 = 2 * CH * hf
                    ps = pspool.tile([128, 2 * CH], f32, tag="io", name="m2ps")
                    for s in range(2):
                        nc.tensor.matmul(ps[:, CH * s:CH * (s + 1)], wm2f,
                                         st.fT[0][:, h0 + CH * s:h0 + CH * (s + 1)],
                                         start=True, stop=True)
                    relu_bias(hf == 0, st.h1[:, h0:h0 + 2 * CH], ps, bias_h1f)
                # d accumulators: [128 paths, 16 windows x 15 steps], one
                # per group, reused across the 4 quarter-T drains
                # (WAR-protected).
                st.dout = [doutp.tile([128, DOUTW], f32, tag="dout", bufs=2,
                                      name="dout") for _ in range(2)]
                st.outsb = [None, None]
                return st

            def drain(st, q):
                """dout quarter q (steps 15q..15q+14) -> outsb, bias b3 fused."""
                for g in range(2):
                    if st.outsb[g] is None:
                        st.outsb[g] = iop.tile([128, NW16 * T], f32, tag="outsb",
                                               bufs=4, name="outsb")
                    dst = st.outsb[g].rearrange("p (w s) -> p w s", s=T)[
                        :, :, TQ * q:TQ * (q + 1)]
                    if H2BF:
                        src = st.dout[g].rearrange("p (w s) -> p w s", s=TQ)
                    else:
                        src = st.dout[g][:, 0:NW16 * TQ].rearrange(
                            "p (s w) -> p w s", w=NW16)
                    nc.scalar.add(dst, src, bias_d)

            def out_dma(st):
                base = st.sc * SC
                for g in range(2):
                    p0 = base + g * G
                    dst3 = out_d[p0:p0 + G, :].rearrange("(w p) t -> p w t", p=128)
                    src3 = st.outsb[g].rearrange("p (w t) -> p w t", t=T)
                    nc.sync.dma_start(dst3, src3)

            def tiny_d(st, t, h2, hf):
                """Transposed-d matmuls for fused half hf: weight = h2 slice,
                rhs = W3 col; d_t lands partition-major at dout[g][:, col]."""
                tt = t % TQ
                for w in range(8 * hf, 8 * (hf + 1)):
                    ws = slice(128 * w, 128 * (w + 1))
                    if H2BF:
                        col = w * TQ + tt
                        for g, hs in ((0, slice(0, 64)), (1, slice(64, 128))):
                            nc.tensor.matmul(
                                st.dout[g][:, col:col + 1], h2[hs, ws],
                                w3sb[hs, 0:1], start=True, stop=True,
                                skip_group_check=True)
                    else:
                        # 2-col write tramples w+1 / next-step slot (fixed by
                        # its later real write)
                        col = tt * NW16 + w
                        for g, hs in ((0, slice(0, 64)), (1, slice(64, 128))):
                            nc.tensor.matmul(
                                st.dout[g][:, col:col + 2], h2[hs, ws],
                                w3sb[hs, 0:2], start=True, stop=True,
                                skip_group_check=True)

            def lane_step(st, t):
                # drain the previous quarter's d accumulator first thing, so
                # this step's tiny-d writes (WAR on the same tile) wait as
                # briefly as possible.
                if t % TQ == 0 and t > 0:
                    drain(st, t // TQ - 1)
                # M1: h2 = relu(diag(W2,W2).T @ h1 + b2).  Two fused halves
                # of [128, 1024] (2 psum banks): half the relu ops and half
                # the cross-engine dependency boundaries of per-512 chunks.
                h2 = statep.tile([128, G], h2dt, tag="h2", bufs=6, name="h2")
                for hf in range(2):
                    h0 = 2 * CH * hf
                    ps = pspool.tile([128, 2 * CH], f32, tag="io", name="m1ps")
                    for s in range(2):
                        cs = slice(h0 + CH * s, h0 + CH * (s + 1))
                        nc.tensor.matmul(ps[:, CH * s:CH * (s + 1)], wm1,
                                         st.h1[:, cs], start=True, stop=True)
                    relu_bias(hf == 1, h2[:, h0:h0 + 2 * CH], ps, bias_h2)
                if t < T - 1:
                    # M2: h1_{t+1} = relu(W13diag.T @ h2 + W1f.T @ fT_{t+1} + bias)
                    w1, i1 = divmod(t + 1, KT)
                    if i1 == 0 and w1 + 1 < T // KT:
                        load_fwin(st, w1 + 1)  # prefetch one window ahead
                    st.h1 = statep.tile([128, G], f32r, tag="h1", bufs=6, name="h1")
                    for hf in range(2):
                        h0 = 2 * CH * hf
                        ps = pspool.tile([128, 2 * CH], f32, tag="io", name="m2ps")
                        for s in range(2):
                            cs = slice(h0 + CH * s, h0 + CH * (s + 1))
                            fs = slice(i1 * G + h0 + CH * s,
                                       i1 * G + h0 + CH * (s + 1))
                            nc.tensor.matmul(ps[:, CH * s:CH * (s + 1)], wm2h,
                                             h2[:, cs], start=True, stop=False)
                            nc.tensor.matmul(ps[:, CH * s:CH * (s + 1)], wm2f,
                                             st.fT[w1][:, fs], start=False,
                                             stop=True)
                        relu_bias(hf == 0, st.h1[:, h0:h0 + 2 * CH], ps, bias_h1)
                        # tiny-d right after M2(hf): its dep (relu-h2(hf)) is
                        # already satisfied, so no PE head-of-line blocking.
                        tiny_d(st, t, h2, hf)
                else:
                    for hf in range(2):
                        tiny_d(st, t, h2, hf)

            # The first quad's prepass runs up front; the next quad's is
            # interleaved into this quad's T-loops, one 512-path window at a
            # time. LANES superchunks run step-interleaved for pipeline depth.
            NQD = NSC // LANES
            for s in range(LANES):
                prepass_initial(s)
            for quad in range(NQD):
                lanes = [lane_init(LANES * quad + i) for i in range(LANES)]
                nxts = [LANES * (quad + 1) + i for i in range(LANES)] \
                    if quad + 1 < NQD else []
                nxt_fts = {}
                nunits = len(nxts) * 3 * NW
                gap_r = max(1, (T - 8) // max(nunits, 1))
                for r in range(T + LANES - 1):
                    if r == 0:
                        for si, s in enumerate(nxts):
                            for w in range(NW):
                                nxt_fts[(si, w)] = prepass_load(s, w)
                    for i, st in enumerate(lanes):
                        ti = r - i
                        if 0 <= ti < T:
                            lane_step(st, ti)
                    if nxts and r % gap_r == 0 and r // gap_r < nunits:
                        u = r // gap_r
                        si, j = divmod(u, 3 * NW)
                        k, w = divmod(j, NW)
                        prepass_k(nxts[si], w, nxt_fts[(si, w)], k)
                for st in lanes:
                    drain(st, NQ4 - 1)
                    out_dma(st)

    nc.compile()
    return nc


def _get_nc():
    if "nc" not in _BUILD_CACHE:
        _BUILD_CACHE["nc"] = _build_nc()
    return _BUILD_CACHE["nc"]


def _host_prep(W1, b1, W2, b2, W3, b3):
    f32 = np.float32
    W1 = np.asarray(W1, f32)
    b1 = np.asarray(b1, f32)
    W2 = np.asarray(W2, f32)
    b2 = np.asarray(b2, f32)
    W3 = np.asarray(W3, f32)
    b3 = np.asarray(b3, f32)
    W1f = W1[0:FEAT, :]                    # (5, 64)
    w1d = W1[FEAT, :]                      # (64,)
    W13 = np.outer(W3[:, 0], w1d)          # (64, 64)  h1pre += W13.T @ h2

    wm1 = np.zeros((128, 128), f32)
    wm1[0:64, 0:64] = W2
    wm1[64:128, 64:128] = W2

    wm2h = np.zeros((128, 128), f32)
    wm2h[0:64, 0:64] = W13
    wm2h[64:128, 64:128] = W13

    wm2f = np.zeros((2 * FEAT, 128), f32)
    wm2f[0:FEAT, 0:64] = W1f
    wm2f[FEAT:2 * FEAT, 64:128] = W1f

    w3col = np.zeros((128, 2), f32)
    w3col[0:64, 0] = W3[:, 0]
    w3col[64:128, 0] = W3[:, 0]

    bias_h2 = np.concatenate([b2, b2]).reshape(128, 1)
    h1b = b1 + b3[0] * w1d
    bias_h1 = np.concatenate([h1b, h1b]).reshape(128, 1)
    bias_h1f = np.concatenate([b1, b1]).reshape(128, 1)
    bias_d = np.full((128, 1), b3[0], f32)
    ident = np.eye(128, dtype=f32)

    if H2BF:
        import ml_dtypes
        wm2h = wm2h.astype(ml_dtypes.bfloat16)
        w3col = w3col.astype(ml_dtypes.bfloat16)
    if FBF:
        import ml_dtypes
        wm2f = wm2f.astype(ml_dtypes.bfloat16)

    return dict(wm1=wm1, wm2h=wm2h, wm2f=wm2f, w3col=w3col, ident=ident,
                bias_h2=bias_h2, bias_h1=bias_h1, bias_h1f=bias_h1f,
                bias_d=bias_d)


def _run(inputs, trace=False):
    from concourse.bass_utils import run_bass_kernel_spmd

    features = np.ascontiguousarray(np.asarray(inputs["features"], np.float32))
    shared = _host_prep(inputs["W1"], inputs["b1"], inputs["W2"], inputs["b2"],
                        inputs["W3"], inputs["b3"])
    nc = _get_nc()

    in_maps = []
    for i in range(NCORES):
        m = dict(shared)
        m["features"] = features[i * B:(i + 1) * B].reshape(B, T * FEAT).copy()
        in_maps.append(m)

    res = run_bass_kernel_spmd(nc, in_maps, core_ids=list(range(NCORES)),
                               trace=trace)
    out = np.concatenate([r["deltas"] for r in res.results], axis=0)
    return out, res


def kernel(**inputs):
    out, _ = _run(inputs, trace=False)
    return out


def kernel_traced(**inputs):
    return _run(inputs, trace=True)


# revision 24
# speedup vs baseline: 1.2284x; 1.2284x over previous
"""Trainium2 Bass kernel for BaselineFeedforwardNetwork forward_trajectory.

Math (per path, T=60 sequential steps with scalar delta feedback):
    x_t = [f_t (5), d_{t-1}]                       (6,)
    h1  = relu(x_t @ W1 + b1)                      (64,)
    h2  = relu(h1 @ W2 + b2)                       (64,)
    d_t = h2 @ W3 + b3                             scalar
Output: deltas (N, T).

Kernel structure (per core, B = N/8 = 16384 paths, data-parallel over 8 cores):
  * Feature-major activations: h1/h2 stored [hidden, path]; the scalar
    feedback d never materializes between steps -- it is folded into the
    next step's first layer via the rank-1 factor W13 = W3 (outer) w1d:
        h1pre_{t+1} = W1f.T @ fT_{t+1} + W13.T @ h2T_t + (b1 + b3*w1d)
  * Two batch groups stacked on 128 partitions (block-diagonal weights) so
    every big matmul/relu uses the full 128-lane width. Per step per
    superchunk (4096 paths = 2048 stacked cols): M1 (W2 layer, 2048 cols),
    M2 = W13-feedback + W1f-features accumulated (2 x 2048 cols). That is
    the PE streaming floor: h1pre and h2pre each saturate the PE's
    128-outputs-per-column bandwidth.
  * d_t extraction costs ~0 streaming: 32 tiny 1-column matmuls per step
    use h2 itself as the stationary weight ([64, 128] slice per 128-path
    window) and W3 as the 1-column rhs, so d lands PARTITION-major
    ([path, step] exactly as the output wants) in 4 persistent PSUM banks
    (2 groups x 2 half-T banks, col = window*30 + t%30). Two drains per
    superchunk (bias b3 fused) -> SBUF, then one 3D-AP DMA per group
    writes deltas directly. No band matmul, no dstage, no output-side
    transposes.
  * h2 is bf16 (h1 stays f32r): enables the 1-col tiny matmuls (f32r
    requires even column counts) and 2x DVE throughput on the h2 relu;
    weights touching h2 (wm2h) are bf16 to match. End-to-end error vs the
    fp32 reference stays ~1e-3 (the recurrence is contractive).
  * Features are transposed on device (PE transpose via identity) into a
    DRAM staging buffer, then streamed as [10, paths] KT-step windows with
    one 3D-access-pattern DMA per window. The next superchunk's prepass is
    interleaved into the T-loop so DMA hides under compute.
  * PSUM budget: 4 io banks (M1/M2 chunk pipeline) + 4 d-accumulator banks.
"""

import os

import numpy as np

N, T, FEAT, H = 131072, 60, 5, 64
NCORES = 8
B = N // NCORES            # 16384 paths per core
SC = int(os.environ.get("K_SC", "4096"))   # paths per superchunk
NSC = B // SC              # superchunks
G = SC // 2                # paths per group (2 groups per superchunk)
CH = 512                   # matmul rhs chunk (fp32 PSUM bank limit)
NCH = G // CH              # chunks per group
IOBUFS = int(os.environ.get("K_IOBUFS", "4"))
KT_ENV = int(os.environ.get("K_KT", "3"))
H2BF = os.environ.get("K_H2DT", "bf16") == "bf16"  # h2/wm2h/w3 in bf16
FBF = os.environ.get("K_FDT", "bf16") == "bf16"    # features/fstage/wm2f in bf16
LANES = int(os.environ.get("K_LANES", "2"))        # interleaved superchunk T-loops
NW16 = G // 128            # 128-path windows per group (16)
TQ = 15                    # steps per d-accumulator drain (quarter-T)
NQ4 = T // TQ              # drains per superchunk per group (4)

_BUILD_CACHE = {}


def _build_nc():
    import concourse.bass as bass  # noqa: F401
    import concourse.mybir as mybir
    import concourse.tile as tile
    from concourse import bacc

    f32 = mybir.dt.float32
    f32r = mybir.dt.float32r
    bf16 = mybir.dt.bfloat16
    h2dt = bf16 if H2BF else f32r
    fdt = bf16 if FBF else f32r
    Relu = mybir.ActivationFunctionType.Relu
    add_op = mybir.AluOpType.add
    max_op = mybir.AluOpType.max

    nc = bacc.Bacc("TRN2", target_bir_lowering=False, debug=False)

    feats = nc.dram_tensor("features", [B, T * FEAT], f32, kind="ExternalInput")
    wm1_d = nc.dram_tensor("wm1", [128, 128], f32r, kind="ExternalInput")
    wm2h_d = nc.dram_tensor("wm2h", [128, 128], h2dt, kind="ExternalInput")
    wm2f_d = nc.dram_tensor("wm2f", [2 * FEAT, 128], fdt, kind="ExternalInput")
    w3_d = nc.dram_tensor("w3col", [128, 2], h2dt, kind="ExternalInput")
    ident_d = nc.dram_tensor("ident", [128, 128], f32, kind="ExternalInput")
    bias_h2_d = nc.dram_tensor("bias_h2", [128, 1], f32, kind="ExternalInput")
    bias_h1_d = nc.dram_tensor("bias_h1", [128, 1], f32, kind="ExternalInput")
    bias_h1f_d = nc.dram_tensor("bias_h1f", [128, 1], f32, kind="ExternalInput")
    bias_d_d = nc.dram_tensor("bias_d", [128, 1], f32, kind="ExternalInput")
    out_d = nc.dram_tensor("deltas", [B, T], f32, kind="ExternalOutput")

    # d-accumulator width: 16 windows x 15 steps; f32r tiny matmuls need a
    # 2-col write (trample-then-fix), bf16 writes 1 col.
    DOUTW = NW16 * TQ + (2 if not H2BF else 0)

    with tile.TileContext(nc) as tc:
        with (
            tc.tile_pool(name="constp", bufs=1) as constp,
            tc.tile_pool(name="iop", bufs=3) as iop,
            tc.tile_pool(name="statep", bufs=2) as statep,
            tc.tile_pool(name="pspool", bufs=IOBUFS, space="PSUM") as pspool,
            tc.tile_pool(name="doutp", bufs=4, space="PSUM") as doutp,
            tc.tile_pool(name="dramp", bufs=1, space="DRAM") as dramp,
        ):
            wm1 = constp.tile_from(wm1_d[:, :], name="wm1_sb")
            wm2h = constp.tile_from(wm2h_d[:, :], name="wm2h_sb")
            wm2f = constp.tile_from(wm2f_d[:, :], name="wm2f_sb")
            w3sb = constp.tile_from(w3_d[:, :], name="w3_sb")
            ident = constp.tile_from(ident_d[:, :], name="ident_sb")
            bias_h2 = constp.tile_from(bias_h2_d[:, :], name="bias_h2_sb")
            bias_h1 = constp.tile_from(bias_h1_d[:, :], name="bias_h1_sb")
            bias_h1f = constp.tile_from(bias_h1f_d[:, :], name="bias_h1f_sb")
            bias_d = constp.tile_from(bias_d_d[:, :], name="bias_d_sb")

            fstage = dramp.tile([T * FEAT, B], fdt, name="fstage")

            def relu_bias(engine_is_act, dst, src, bias_ap):
                if engine_is_act:
                    nc.scalar.activation(dst, src, Relu, bias=bias_ap)
                else:
                    nc.vector.tensor_scalar(dst, src, bias_ap, 0.0,
                                            add_op, max_op)

            KT = KT_ENV  # steps per batched fT window DMA
            assert T % KT == 0
            NW = SC // 512  # 512-path windows per superchunk (prepass units)

            def prepass_load(sc, w):
                """DMA 512 paths' raw features into SBUF; return j-slices."""
                base = sc * SC
                p0 = base + w * 512
                ft = iop.tile([128, 4 * T * FEAT], f32, tag="Ftile", bufs=8,
                              name="Ftile")
                src3 = feats[p0:p0 + 512, :].rearrange("(j l) c -> l j c", l=128)
                dst3 = ft.rearrange("l (j c) -> l j c", j=4)
                nc.sync.dma_start(dst3, src3)
                return ft

            def prepass_k(sc, w, ft, k):
                """Transpose rows [100k, 100k+100) of window w into fstage."""
                base = sc * SC
                p0 = base + w * 512
                ps_tr = pspool.tile([128, 512], f32, tag="io", name="ps_tr")
                for j in range(4):
                    nc.tensor.transpose(
                        ps_tr[0:100, 128 * j:128 * (j + 1)],
                        ft[:, T * FEAT * j + 100 * k:T * FEAT * j + 100 * (k + 1)],
                        ident,
                    )
                stg = iop.tile([128, 512], fdt, tag="stg", name="stg")
                nc.scalar.copy(stg[0:100, :], ps_tr[0:100, :])
                nc.sync.dma_start(
                    fstage[100 * k:100 * (k + 1), p0:p0 + 512], stg[0:100, :]
                )

            def prepass_span(sc, w0, w1):
                """Stage 512-path windows [w0, w1) of sc into fstage."""
                fts = {w: prepass_load(sc, w) for w in range(w0, w1)}
                for k in range(3):
                    for w in range(w0, w1):
                        prepass_k(sc, w, fts[w], k)

            def prepass_initial(sc):
                """First superchunk: all window DMAs up front, transposes
                k-major across ALL windows so the T-loop (which consumes
                fstage rows in step order) can start after ~1/3 of the work.
                """
                fts = {w: prepass_load(sc, w) for w in range(NW)}
                for k in range(3):
                    for w in range(NW):
                        prepass_k(sc, w, fts[w], k)

            class Lane:
                pass

            def load_fwin(st, w):
                """Load fT for steps [KT*w, KT*w + KT): rows 0-4 group A, 5-9 B."""
                t0 = KT * w
                fTbig = iop.tile([2 * FEAT, KT * G], fdt, tag="fTbig", bufs=6,
                                 name="fTbig")
                for half, col in ((0, st.colA), (1, st.colB)):
                    src = fstage[FEAT * t0:FEAT * (t0 + KT), col]
                    src3 = src.rearrange("(k c) n -> c k n", c=FEAT)
                    dst3 = fTbig[FEAT * half:FEAT * (half + 1), :].rearrange(
                        "c (k n) -> c k n", n=G)
                    nc.sync.dma_start(dst3, src3)
                st.fT[w] = fTbig
                st.fT.pop(w - 2, None)

            def lane_init(sc):
                st = Lane()
                st.sc = sc
                base = sc * SC
                st.colA = slice(base, base + G)
                st.colB = slice(base + G, base + SC)
                st.fT = {}
                load_fwin(st, 0)
                load_fwin(st, 1)  # prefetch one window ahead
                st.h1 = statep.tile([128, G], f32r, tag="h1", bufs=6, name="h1")
                for c in range(NCH):
                    cs = slice(CH * c, CH * (c + 1))
                    ps = pspool.tile([128, CH], f32, tag="io", name="m2ps")
                    nc.tensor.matmul(ps, wm2f, st.fT[0][:, cs], start=True, stop=True)
                    relu_bias(c % 2 == 0, st.h1[:, cs], ps, bias_h1f)
                # d accumulators: [128 paths, 16 windows x 15 steps], one
                # per group, reused across the 4 quarter-T drains
                # (WAR-protected).
                st.dout = [doutp.tile([128, DOUTW], f32, tag="dout", bufs=4,
                                      name="dout") for _ in range(2)]
                st.outsb = [None, None]
                return st

            def drain(st, q):
                """dout quarter q (steps 15q..15q+14) -> outsb, bias b3 fused."""
                for g in range(2):
                    if st.outsb[g] is None:
                        st.outsb[g] = iop.tile([128, NW16 * T], f32, tag="outsb",
                                               bufs=4, name="outsb")
                    dst = st.outsb[g].rearrange("p (w s) -> p w s", s=T)[
                        :, :, TQ * q:TQ * (q + 1)]
                    if H2BF:
                        src = st.dout[g].rearrange("p (w s) -> p w s", s=TQ)
                    else:
                        src = st.dout[g][:, 0:NW16 * TQ].rearrange(
                            "p (s w) -> p w s", w=NW16)
                    nc.scalar.add(dst, src, bias_d)

            def out_dma(st):
                base = st.sc * SC
                for g in range(2):
                    p0 = base + g * G
                    dst3 = out_d[p0:p0 + G, :].rearrange("(w p) t -> p w t", p=128)
                    src3 = st.outsb[g].rearrange("p (w t) -> p w t", t=T)
                    nc.sync.dma_start(dst3, src3)

            def tiny_d(st, t, h2, c):
                """Transposed-d matmuls for chunk c: weight = h2 slice, rhs =
                W3 col; d_t lands partition-major at dout[g][:, col]."""
                tt = t % TQ
                for w in range(4 * c, 4 * (c + 1)):
                    ws = slice(128 * w, 128 * (w + 1))
                    if H2BF:
                        col = w * TQ + tt
                        for g, hs in ((0, slice(0, 64)), (1, slice(64, 128))):
                            nc.tensor.matmul(
                                st.dout[g][:, col:col + 1], h2[hs, ws],
                                w3sb[hs, 0:1], start=True, stop=True,
                                skip_group_check=True)
                    else:
                        # 2-col write tramples w+1 / next-step slot (fixed by
                        # its later real write)
                        col = tt * NW16 + w
                        for g, hs in ((0, slice(0, 64)), (1, slice(64, 128))):
                            nc.tensor.matmul(
                                st.dout[g][:, col:col + 2], h2[hs, ws],
                                w3sb[hs, 0:2], start=True, stop=True,
                                skip_group_check=True)

            def lane_step(st, t):
                # drain the previous quarter's d accumulator first thing, so
                # this step's tiny-d writes (WAR on the same tile) wait as
                # briefly as possible.
                if t % TQ == 0 and t > 0:
                    drain(st, t // TQ - 1)
                # M1: h2 = relu(diag(W2,W2).T @ h1 + b2)
                h2 = statep.tile([128, G], h2dt, tag="h2", bufs=6, name="h2")
                for c in range(NCH):
                    cs = slice(CH * c, CH * (c + 1))
                    ps = pspool.tile([128, CH], f32, tag="io", name="m1ps")
                    nc.tensor.matmul(ps, wm1, st.h1[:, cs], start=True, stop=True)
                    relu_bias(c % 2 == 0, h2[:, cs], ps, bias_h2)
                if t < T - 1:
                    # M2: h1_{t+1} = relu(W13diag.T @ h2 + W1f.T @ fT_{t+1} + bias)
                    w1, i1 = divmod(t + 1, KT)
                    if i1 == 0 and w1 + 1 < T // KT:
                        load_fwin(st, w1 + 1)  # prefetch one window ahead
                    st.h1 = statep.tile([128, G], f32r, tag="h1", bufs=6, name="h1")
                    for c in range(NCH):
                        cs = slice(CH * c, CH * (c + 1))
                        fs = slice(i1 * G + CH * c, i1 * G + CH * (c + 1))
                        ps = pspool.tile([128, CH], f32, tag="io", name="m2ps")
                        nc.tensor.matmul(ps, wm2h, h2[:, cs], start=True, stop=False)
                        nc.tensor.matmul(ps, wm2f, st.fT[w1][:, fs], start=False,
                                         stop=True)
                        relu_bias(c % 2 == 1, st.h1[:, cs], ps, bias_h1)
                        # tiny-d right after M2(c): its dep (relu-h2(c)) is
                        # already satisfied, so no PE head-of-line blocking.
                        tiny_d(st, t, h2, c)
                else:
                    for c in range(NCH):
                        tiny_d(st, t, h2, c)

            # The first quad's prepass runs up front; the next quad's is
            # interleaved into this quad's T-loops, one 512-path window at a
            # time. LANES superchunks run step-interleaved for pipeline depth.
            NQD = NSC // LANES
            for s in range(LANES):
                prepass_initial(s)
            for quad in range(NQD):
                lanes = [lane_init(LANES * quad + i) for i in range(LANES)]
                nxts = [LANES * (quad + 1) + i for i in range(LANES)] \
                    if quad + 1 < NQD else []
                nxt_fts = {}
                nunits = len(nxts) * 3 * NW
                gap_r = max(1, (T - 8) // max(nunits, 1))
                for r in range(T + LANES - 1):
                    if r == 0:
                        for si, s in enumerate(nxts):
                            for w in range(NW):
                                nxt_fts[(si, w)] = prepass_load(s, w)
                    for i, st in enumerate(lanes):
                        ti = r - i
                        if 0 <= ti < T:
                            lane_step(st, ti)
                    if nxts and r % gap_r == 0 and r // gap_r < nunits:
                        u = r // gap_r
                        si, j = divmod(u, 3 * NW)
                        k, w = divmod(j, NW)
                        prepass_k(nxts[si], w, nxt_fts[(si, w)], k)
                for st in lanes:
                    drain(st, NQ4 - 1)
                    out_dma(st)

    nc.compile()
    return nc


def _get_nc():
    if "nc" not in _BUILD_CACHE:
        _BUILD_CACHE["nc"] = _build_nc()
    return _BUILD_CACHE["nc"]


def _host_prep(W1, b1, W2, b2, W3, b3):
    f32 = np.float32
    W1 = np.asarray(W1, f32)
    b1 = np.asarray(b1, f32)
    W2 = np.asarray(W2, f32)
    b2 = np.asarray(b2, f32)
    W3 = np.asarray(W3, f32)
    b3 = np.asarray(b3, f32)
    W1f = W1[0:FEAT, :]                    # (5, 64)
    w1d = W1[FEAT, :]                      # (64,)
    W13 = np.outer(W3[:, 0], w1d)          # (64, 64)  h1pre += W13.T @ h2

    wm1 = np.zeros((128, 128), f32)
    wm1[0:64, 0:64] = W2
    wm1[64:128, 64:128] = W2

    wm2h = np.zeros((128, 128), f32)
    wm2h[0:64, 0:64] = W13
    wm2h[64:128, 64:128] = W13

    wm2f = np.zeros((2 * FEAT, 128), f32)
    wm2f[0:FEAT, 0:64] = W1f
    wm2f[FEAT:2 * FEAT, 64:128] = W1f

    w3col = np.zeros((128, 2), f32)
    w3col[0:64, 0] = W3[:, 0]
    w3col[64:128, 0] = W3[:, 0]

    bias_h2 = np.concatenate([b2, b2]).reshape(128, 1)
    h1b = b1 + b3[0] * w1d
    bias_h1 = np.concatenate([h1b, h1b]).reshape(128, 1)
    bias_h1f = np.concatenate([b1, b1]).reshape(128, 1)
    bias_d = np.full((128, 1), b3[0], f32)
    ident = np.eye(128, dtype=f32)

    if H2BF:
        import ml_dtypes
        wm2h = wm2h.astype(ml_dtypes.bfloat16)
        w3col = w3col.astype(ml_dtypes.bfloat16)
    if FBF:
        import ml_dtypes
        wm2f = wm2f.astype(ml_dtypes.bfloat16)

    return dict(wm1=wm1, wm2h=wm2h, wm2f=wm2f, w3col=w3col, ident=ident,
                bias_h2=bias_h2, bias_h1=bias_h1, bias_h1f=bias_h1f,
                bias_d=bias_d)


def _run(inputs, trace=False):
    from concourse.bass_utils import run_bass_kernel_spmd

    features = np.ascontiguousarray(np.asarray(inputs["features"], np.float32))
    shared = _host_prep(inputs["W1"], inputs["b1"], inputs["W2"], inputs["b2"],
                        inputs["W3"], inputs["b3"])
    nc = _get_nc()

    in_maps = []
    for i in range(NCORES):
        m = dict(shared)
        m["features"] = features[i * B:(i + 1) * B].reshape(B, T * FEAT).copy()
        in_maps.append(m)

    res = run_bass_kernel_spmd(nc, in_maps, core_ids=list(range(NCORES)),
                               trace=trace)
    out = np.concatenate([r["deltas"] for r in res.results], axis=0)
    return out, res


def kernel(**inputs):
    out, _ = _run(inputs, trace=False)
    return out


def kernel_traced(**inputs):
    return _run(inputs, trace=True)


# revision 25
# speedup vs baseline: 1.2611x; 1.0266x over previous
"""Trainium2 Bass kernel for BaselineFeedforwardNetwork forward_trajectory.

Math (per path, T=60 sequential steps with scalar delta feedback):
    x_t = [f_t (5), d_{t-1}]                       (6,)
    h1  = relu(x_t @ W1 + b1)                      (64,)
    h2  = relu(h1 @ W2 + b2)                       (64,)
    d_t = h2 @ W3 + b3                             scalar
Output: deltas (N, T).

Kernel structure (per core, B = N/8 = 16384 paths, data-parallel over 8 cores):
  * Feature-major activations: h1/h2 stored [hidden, path]; the scalar
    feedback d never materializes between steps -- it is folded into the
    next step's first layer via the rank-1 factor W13 = W3 (outer) w1d:
        h1pre_{t+1} = W1f.T @ fT_{t+1} + W13.T @ h2T_t + (b1 + b3*w1d)
  * Two batch groups stacked on 128 partitions (block-diagonal weights) so
    every big matmul/relu uses the full 128-lane width. Per step per
    superchunk (4096 paths = 2048 stacked cols): M1 (W2 layer, 2048 cols),
    M2 = W13-feedback + W1f-features accumulated (2 x 2048 cols). That is
    the PE streaming floor: h1pre and h2pre each saturate the PE's
    128-outputs-per-column bandwidth.
  * d_t extraction costs ~0 streaming: 32 tiny 1-column matmuls per step
    use h2 itself as the stationary weight ([64, 128] slice per 128-path
    window) and W3 as the 1-column rhs, so d lands PARTITION-major
    ([path, step] exactly as the output wants) in 4 persistent PSUM banks
    (2 groups x 2 half-T banks, col = window*30 + t%30). Two drains per
    superchunk (bias b3 fused) -> SBUF, then one 3D-AP DMA per group
    writes deltas directly. No band matmul, no dstage, no output-side
    transposes.
  * h2 is bf16 (h1 stays f32r): enables the 1-col tiny matmuls (f32r
    requires even column counts) and 2x DVE throughput on the h2 relu;
    weights touching h2 (wm2h) are bf16 to match. End-to-end error vs the
    fp32 reference stays ~1e-3 (the recurrence is contractive).
  * Features are transposed on device (PE transpose via identity) into a
    DRAM staging buffer, then streamed as [10, paths] KT-step windows with
    one 3D-access-pattern DMA per window. The next superchunk's prepass is
    interleaved into the T-loop so DMA hides under compute.
  * PSUM budget: 4 io banks (M1/M2 chunk pipeline) + 4 d-accumulator banks.
"""

import os

import numpy as np

N, T, FEAT, H = 131072, 60, 5, 64
NCORES = 8
B = N // NCORES            # 16384 paths per core
SC = int(os.environ.get("K_SC", "4096"))   # paths per superchunk
NSC = B // SC              # superchunks
G = SC // 2                # paths per group (2 groups per superchunk)
CH = 512                   # matmul rhs chunk (fp32 PSUM bank limit)
NCH = G // CH              # chunks per group
IOBUFS = int(os.environ.get("K_IOBUFS", "4"))
KT_ENV = int(os.environ.get("K_KT", "3"))
H2BF = os.environ.get("K_H2DT", "bf16") == "bf16"  # h2/wm2h/w3 in bf16
FBF = os.environ.get("K_FDT", "bf16") == "bf16"    # features/fstage/wm2f in bf16
LANES = int(os.environ.get("K_LANES", "2"))        # interleaved superchunk T-loops
NW16 = G // 128            # 128-path windows per group (16)
TQ = 15                    # steps per d-accumulator drain (quarter-T)
NQ4 = T // TQ              # drains per superchunk per group (4)

_BUILD_CACHE = {}


def _build_nc():
    import concourse.bass as bass  # noqa: F401
    import concourse.mybir as mybir
    import concourse.tile as tile
    from concourse import bacc

    f32 = mybir.dt.float32
    f32r = mybir.dt.float32r
    bf16 = mybir.dt.bfloat16
    h2dt = bf16 if H2BF else f32r
    fdt = bf16 if FBF else f32r
    Relu = mybir.ActivationFunctionType.Relu
    add_op = mybir.AluOpType.add
    max_op = mybir.AluOpType.max

    nc = bacc.Bacc("TRN2", target_bir_lowering=False, debug=False)

    feats = nc.dram_tensor("features", [B, T * FEAT], f32, kind="ExternalInput")
    wm1_d = nc.dram_tensor("wm1", [128, 128], f32r, kind="ExternalInput")
    wm2h_d = nc.dram_tensor("wm2h", [128, 128], h2dt, kind="ExternalInput")
    wm2f_d = nc.dram_tensor("wm2f", [2 * FEAT, 128], fdt, kind="ExternalInput")
    w3_d = nc.dram_tensor("w3col", [128, 2], h2dt, kind="ExternalInput")
    ident_d = nc.dram_tensor("ident", [128, 128], f32, kind="ExternalInput")
    bias_h2_d = nc.dram_tensor("bias_h2", [128, 1], f32, kind="ExternalInput")
    bias_h1_d = nc.dram_tensor("bias_h1", [128, 1], f32, kind="ExternalInput")
    bias_h1f_d = nc.dram_tensor("bias_h1f", [128, 1], f32, kind="ExternalInput")
    bias_d_d = nc.dram_tensor("bias_d", [128, 1], f32, kind="ExternalInput")
    out_d = nc.dram_tensor("deltas", [B, T], f32, kind="ExternalOutput")

    # d-accumulator width: 16 windows x 15 steps; f32r tiny matmuls need a
    # 2-col write (trample-then-fix), bf16 writes 1 col.
    DOUTW = NW16 * TQ + (2 if not H2BF else 0)

    with tile.TileContext(nc) as tc:
        with (
            tc.tile_pool(name="constp", bufs=1) as constp,
            tc.tile_pool(name="iop", bufs=3) as iop,
            tc.tile_pool(name="statep", bufs=2) as statep,
            tc.tile_pool(name="pspool", bufs=IOBUFS, space="PSUM") as pspool,
            tc.tile_pool(name="doutp", bufs=4, space="PSUM") as doutp,
            tc.tile_pool(name="dramp", bufs=1, space="DRAM") as dramp,
        ):
            wm1 = constp.tile_from(wm1_d[:, :], name="wm1_sb")
            wm2h = constp.tile_from(wm2h_d[:, :], name="wm2h_sb")
            wm2f = constp.tile_from(wm2f_d[:, :], name="wm2f_sb")
            w3sb = constp.tile_from(w3_d[:, :], name="w3_sb")
            ident = constp.tile_from(ident_d[:, :], name="ident_sb")
            bias_h2 = constp.tile_from(bias_h2_d[:, :], name="bias_h2_sb")
            bias_h1 = constp.tile_from(bias_h1_d[:, :], name="bias_h1_sb")
            bias_h1f = constp.tile_from(bias_h1f_d[:, :], name="bias_h1f_sb")
            bias_d = constp.tile_from(bias_d_d[:, :], name="bias_d_sb")

            fstage = dramp.tile([T * FEAT, B], fdt, name="fstage")

            def relu_bias(engine_is_act, dst, src, bias_ap):
                if engine_is_act:
                    nc.scalar.activation(dst, src, Relu, bias=bias_ap)
                else:
                    nc.vector.tensor_scalar(dst, src, bias_ap, 0.0,
                                            add_op, max_op)

            KT = KT_ENV  # steps per batched fT window DMA
            assert T % KT == 0
            NW = SC // 512  # 512-path windows per superchunk (prepass units)

            def prepass_load(sc, w):
                """DMA 512 paths' raw features into SBUF; return j-slices."""
                base = sc * SC
                p0 = base + w * 512
                ft = iop.tile([128, 4 * T * FEAT], f32, tag="Ftile", bufs=8,
                              name="Ftile")
                src3 = feats[p0:p0 + 512, :].rearrange("(j l) c -> l j c", l=128)
                dst3 = ft.rearrange("l (j c) -> l j c", j=4)
                nc.sync.dma_start(dst3, src3)
                return ft

            def prepass_k(sc, w, ft, k):
                """Transpose rows [100k, 100k+100) of window w into fstage."""
                base = sc * SC
                p0 = base + w * 512
                ps_tr = pspool.tile([128, 512], f32, tag="io", name="ps_tr")
                for j in range(4):
                    nc.tensor.transpose(
                        ps_tr[0:100, 128 * j:128 * (j + 1)],
                        ft[:, T * FEAT * j + 100 * k:T * FEAT * j + 100 * (k + 1)],
                        ident,
                    )
                stg = iop.tile([128, 512], fdt, tag="stg", name="stg")
                if (w + k) % 2 == 0:
                    nc.scalar.copy(stg[0:100, :], ps_tr[0:100, :])
                else:
                    nc.vector.tensor_copy(stg[0:100, :], ps_tr[0:100, :])
                nc.sync.dma_start(
                    fstage[100 * k:100 * (k + 1), p0:p0 + 512], stg[0:100, :]
                )

            def prepass_span(sc, w0, w1):
                """Stage 512-path windows [w0, w1) of sc into fstage."""
                fts = {w: prepass_load(sc, w) for w in range(w0, w1)}
                for k in range(3):
                    for w in range(w0, w1):
                        prepass_k(sc, w, fts[w], k)

            def prepass_initial(sc):
                """First superchunk: all window DMAs up front, transposes
                k-major across ALL windows so the T-loop (which consumes
                fstage rows in step order) can start after ~1/3 of the work.
                """
                fts = {w: prepass_load(sc, w) for w in range(NW)}
                for k in range(3):
                    for w in range(NW):
                        prepass_k(sc, w, fts[w], k)

            class Lane:
                pass

            def load_fwin(st, w):
                """Load fT for steps [KT*w, KT*w + KT): rows 0-4 group A, 5-9 B."""
                t0 = KT * w
                fTbig = iop.tile([2 * FEAT, KT * G], fdt, tag="fTbig", bufs=6,
                                 name="fTbig")
                for half, col in ((0, st.colA), (1, st.colB)):
                    src = fstage[FEAT * t0:FEAT * (t0 + KT), col]
                    src3 = src.rearrange("(k c) n -> c k n", c=FEAT)
                    dst3 = fTbig[FEAT * half:FEAT * (half + 1), :].rearrange(
                        "c (k n) -> c k n", n=G)
                    nc.sync.dma_start(dst3, src3)
                st.fT[w] = fTbig
                st.fT.pop(w - 2, None)

            def lane_init(sc):
                st = Lane()
                st.sc = sc
                base = sc * SC
                st.colA = slice(base, base + G)
                st.colB = slice(base + G, base + SC)
                st.fT = {}
                load_fwin(st, 0)
                load_fwin(st, 1)  # prefetch one window ahead
                st.h1 = statep.tile([128, G], f32r, tag="h1", bufs=6, name="h1")
                for c in range(NCH):
                    cs = slice(CH * c, CH * (c + 1))
                    ps = pspool.tile([128, CH], f32, tag="io", name="m2ps")
                    nc.tensor.matmul(ps, wm2f, st.fT[0][:, cs], start=True, stop=True)
                    relu_bias(c % 2 == 0, st.h1[:, cs], ps, bias_h1f)
                # d accumulators: [128 paths, 16 windows x 15 steps], one
                # per group, reused across the 4 quarter-T drains
                # (WAR-protected).
                st.dout = [doutp.tile([128, DOUTW], f32, tag="dout", bufs=4,
                                      name="dout") for _ in range(2)]
                st.outsb = [None, None]
                return st

            def drain(st, q):
                """dout quarter q (steps 15q..15q+14) -> outsb, bias b3 fused."""
                for g in range(2):
                    if st.outsb[g] is None:
                        st.outsb[g] = iop.tile([128, NW16 * T], f32, tag="outsb",
                                               bufs=4, name="outsb")
                    dst = st.outsb[g].rearrange("p (w s) -> p w s", s=T)[
                        :, :, TQ * q:TQ * (q + 1)]
                    if H2BF:
                        src = st.dout[g].rearrange("p (w s) -> p w s", s=TQ)
                    else:
                        src = st.dout[g][:, 0:NW16 * TQ].rearrange(
                            "p (s w) -> p w s", w=NW16)
                    nc.scalar.add(dst, src, bias_d)

            def out_dma(st):
                base = st.sc * SC
                for g in range(2):
                    p0 = base + g * G
                    dst3 = out_d[p0:p0 + G, :].rearrange("(w p) t -> p w t", p=128)
                    src3 = st.outsb[g].rearrange("p (w t) -> p w t", t=T)
                    nc.sync.dma_start(dst3, src3)

            def tiny_d(st, t, h2, c):
                """Transposed-d matmuls for chunk c: weight = h2 slice, rhs =
                W3 col; d_t lands partition-major at dout[g][:, col]."""
                tt = t % TQ
                for w in range(4 * c, 4 * (c + 1)):
                    ws = slice(128 * w, 128 * (w + 1))
                    if H2BF:
                        col = w * TQ + tt
                        for g, hs in ((0, slice(0, 64)), (1, slice(64, 128))):
                            nc.tensor.matmul(
                                st.dout[g][:, col:col + 1], h2[hs, ws],
                                w3sb[hs, 0:1], start=True, stop=True,
                                skip_group_check=True)
                    else:
                        # 2-col write tramples w+1 / next-step slot (fixed by
                        # its later real write)
                        col = tt * NW16 + w
                        for g, hs in ((0, slice(0, 64)), (1, slice(64, 128))):
                            nc.tensor.matmul(
                                st.dout[g][:, col:col + 2], h2[hs, ws],
                                w3sb[hs, 0:2], start=True, stop=True,
                                skip_group_check=True)

            def lane_step(st, t):
                # drain the previous quarter's d accumulator first thing, so
                # this step's tiny-d writes (WAR on the same tile) wait as
                # briefly as possible.
                if t % TQ == 0 and t > 0:
                    drain(st, t // TQ - 1)
                # M1: h2 = relu(diag(W2,W2).T @ h1 + b2)
                h2 = statep.tile([128, G], h2dt, tag="h2", bufs=6, name="h2")
                for c in range(NCH):
                    cs = slice(CH * c, CH * (c + 1))
                    ps = pspool.tile([128, CH], f32, tag="io", name="m1ps")
                    nc.tensor.matmul(ps, wm1, st.h1[:, cs], start=True, stop=True)
                    relu_bias(c % 2 == 0, h2[:, cs], ps, bias_h2)
                if t < T - 1:
                    # M2: h1_{t+1} = relu(W13diag.T @ h2 + W1f.T @ fT_{t+1} + bias)
                    w1, i1 = divmod(t + 1, KT)
                    if i1 == 0 and w1 + 1 < T // KT:
                        load_fwin(st, w1 + 1)  # prefetch one window ahead
                    st.h1 = statep.tile([128, G], f32r, tag="h1", bufs=6, name="h1")
                    for c in range(NCH):
                        cs = slice(CH * c, CH * (c + 1))
                        fs = slice(i1 * G + CH * c, i1 * G + CH * (c + 1))
                        ps = pspool.tile([128, CH], f32, tag="io", name="m2ps")
                        nc.tensor.matmul(ps, wm2h, h2[:, cs], start=True, stop=False)
                        nc.tensor.matmul(ps, wm2f, st.fT[w1][:, fs], start=False,
                                         stop=True)
                        relu_bias(c % 2 == 1, st.h1[:, cs], ps, bias_h1)
                        # tiny-d right after M2(c): its dep (relu-h2(c)) is
                        # already satisfied, so no PE head-of-line blocking.
                        tiny_d(st, t, h2, c)
                else:
                    for c in range(NCH):
                        tiny_d(st, t, h2, c)

            # The first quad's prepass runs up front; the next quad's is
            # interleaved into this quad's T-loops, one 512-path window at a
            # time. LANES superchunks run step-interleaved for pipeline depth.
            NQD = NSC // LANES
            for s in range(LANES):
                prepass_initial(s)
            for quad in range(NQD):
                lanes = [lane_init(LANES * quad + i) for i in range(LANES)]
                nxts = [LANES * (quad + 1) + i for i in range(LANES)] \
                    if quad + 1 < NQD else []
                nspan = len(nxts) * NW
                gap_r = max(1, (T - 8) // max(nspan, 1))
                for r in range(T + LANES - 1):
                    for i, st in enumerate(lanes):
                        ti = r - i
                        if 0 <= ti < T:
                            lane_step(st, ti)
                    if nxts and r % gap_r == 0 and r // gap_r < nspan:
                        j = r // gap_r
                        prepass_span(nxts[j // NW], j % NW, j % NW + 1)
                for st in lanes:
                    drain(st, NQ4 - 1)
                    out_dma(st)

    nc.compile()
    return nc


def _get_nc():
    if "nc" not in _BUILD_CACHE:
        _BUILD_CACHE["nc"] = _build_nc()
    return _BUILD_CACHE["nc"]


def _host_prep(W1, b1, W2, b2, W3, b3):
    f32 = np.float32
    W1 = np.asarray(W1, f32)
    b1 = np.asarray(b1, f32)
    W2 = np.asarray(W2, f32)
    b2 = np.asarray(b2, f32)
    W3 = np.asarray(W3, f32)
    b3 = np.asarray(b3, f32)
    W1f = W1[0:FEAT, :]                    # (5, 64)
    w1d = W1[FEAT, :]                      # (64,)
    W13 = np.outer(W3[:, 0], w1d)          # (64, 64)  h1pre += W13.T @ h2

    wm1 = np.zeros((128, 128), f32)
    wm1[0:64, 0:64] = W2
    wm1[64:128, 64:128] = W2

    wm2h = np.zeros((128, 128), f32)
    wm2h[0:64, 0:64] = W13
    wm2h[64:128, 64:128] = W13

    wm2f = np.zeros((2 * FEAT, 128), f32)
    wm2f[0:FEAT, 0:64] = W1f
    wm2f[FEAT:2 * FEAT, 64:128] = W1f

    w3col = np.zeros((128, 2), f32)
    w3col[0:64, 0] = W3[:, 0]
    w3col[64:128, 0] = W3[:, 0]

    bias_h2 = np.concatenate([b2, b2]).reshape(128, 1)
    h1b = b1 + b3[0] * w1d
    bias_h1 = np.concatenate([h1b, h1b]).reshape(128, 1)
    bias_h1f = np.concatenate([b1, b1]).reshape(128, 1)
    bias_d = np.full((128, 1), b3[0], f32)
    ident = np.eye(128, dtype=f32)

    if H2BF:
        import ml_dtypes
        wm2h = wm2h.astype(ml_dtypes.bfloat16)
        w3col = w3col.astype(ml_dtypes.bfloat16)
    if FBF:
        import ml_dtypes
        wm2f = wm2f.astype(ml_dtypes.bfloat16)

    return dict(wm1=wm1, wm2h=wm2h, wm2f=wm2f, w3col=w3col, ident=ident,
                bias_h2=bias_h2, bias_h1=bias_h1, bias_h1f=bias_h1f,
                bias_d=bias_d)


def _run(inputs, trace=False):
    from concourse.bass_utils import run_bass_kernel_spmd

    features = np.ascontiguousarray(np.asarray(inputs["features"], np.float32))
    shared = _host_prep(inputs["W1"], inputs["b1"], inputs["W2"], inputs["b2"],
                        inputs["W3"], inputs["b3"])
    nc = _get_nc()

    in_maps = []
    for i in range(NCORES):
        m = dict(shared)
        m["features"] = features[i * B:(i + 1) * B].reshape(B, T * FEAT).copy()
        in_maps.append(m)

    res = run_bass_kernel_spmd(nc, in_maps, core_ids=list(range(NCORES)),
                               trace=trace)
    out = np.concatenate([r["deltas"] for r in res.results], axis=0)
    return out, res


def kernel(**inputs):
    out, _ = _run(inputs, trace=False)
    return out


def kernel_traced(**inputs):
    return _run(inputs, trace=True)


# revision 26
# speedup vs baseline: 1.2838x; 1.0180x over previous
"""Trainium2 Bass kernel for BaselineFeedforwardNetwork forward_trajectory.

Math (per path, T=60 sequential steps with scalar delta feedback):
    x_t = [f_t (5), d_{t-1}]                       (6,)
    h1  = relu(x_t @ W1 + b1)                      (64,)
    h2  = relu(h1 @ W2 + b2)                       (64,)
    d_t = h2 @ W3 + b3                             scalar
Output: deltas (N, T).

Kernel structure (per core, B = N/8 = 16384 paths, data-parallel over 8 cores):
  * Feature-major activations: h1/h2 stored [hidden, path]; the scalar
    feedback d never materializes between steps -- it is folded into the
    next step's first layer via the rank-1 factor W13 = W3 (outer) w1d:
        h1pre_{t+1} = W1f.T @ fT_{t+1} + W13.T @ h2T_t + (b1 + b3*w1d)
  * Two batch groups stacked on 128 partitions (block-diagonal weights) so
    every big matmul/relu uses the full 128-lane width. Per step per
    superchunk (4096 paths = 2048 stacked cols): M1 (W2 layer, 2048 cols),
    M2 = W13-feedback + W1f-features accumulated (2 x 2048 cols). That is
    the PE streaming floor: h1pre and h2pre each saturate the PE's
    128-outputs-per-column bandwidth.
  * d_t extraction costs ~0 streaming: 32 tiny 1-column matmuls per step
    use h2 itself as the stationary weight ([64, 128] slice per 128-path
    window) and W3 as the 1-column rhs, so d lands PARTITION-major
    ([path, step] exactly as the output wants) in 4 persistent PSUM banks
    (2 groups x 2 half-T banks, col = window*30 + t%30). Two drains per
    superchunk (bias b3 fused) -> SBUF, then one 3D-AP DMA per group
    writes deltas directly. No band matmul, no dstage, no output-side
    transposes.
  * h2 is bf16 (h1 stays f32r): enables the 1-col tiny matmuls (f32r
    requires even column counts) and 2x DVE throughput on the h2 relu;
    weights touching h2 (wm2h) are bf16 to match. End-to-end error vs the
    fp32 reference stays ~1e-3 (the recurrence is contractive).
  * Features are transposed on device (PE transpose via identity) into a
    DRAM staging buffer, then streamed as [10, paths] KT-step windows with
    one 3D-access-pattern DMA per window. The next superchunk's prepass is
    interleaved into the T-loop so DMA hides under compute.
  * PSUM budget: 4 io banks (M1/M2 chunk pipeline) + 4 d-accumulator banks.
"""

import os

import numpy as np

N, T, FEAT, H = 131072, 60, 5, 64
NCORES = 8
B = N // NCORES            # 16384 paths per core
SC = int(os.environ.get("K_SC", "4096"))   # paths per superchunk
NSC = B // SC              # superchunks
G = SC // 2                # paths per group (2 groups per superchunk)
CH = 512                   # matmul rhs chunk (fp32 PSUM bank limit)
NCH = G // CH              # chunks per group
IOBUFS = int(os.environ.get("K_IOBUFS", "4"))
KT_ENV = int(os.environ.get("K_KT", "3"))
H2BF = os.environ.get("K_H2DT", "bf16") == "bf16"  # h2/wm2h/w3 in bf16
FBF = os.environ.get("K_FDT", "bf16") == "bf16"    # features/fstage/wm2f in bf16
LANES = int(os.environ.get("K_LANES", "2"))        # interleaved superchunk T-loops
NW16 = G // 128            # 128-path windows per group (16)
TQ = 15                    # steps per d-accumulator drain (quarter-T)
NQ4 = T // TQ              # drains per superchunk per group (4)

_BUILD_CACHE = {}


def _build_nc():
    import concourse.bass as bass  # noqa: F401
    import concourse.mybir as mybir
    import concourse.tile as tile
    from concourse import bacc

    f32 = mybir.dt.float32
    f32r = mybir.dt.float32r
    bf16 = mybir.dt.bfloat16
    h2dt = bf16 if H2BF else f32r
    fdt = bf16 if FBF else f32r
    Relu = mybir.ActivationFunctionType.Relu
    add_op = mybir.AluOpType.add
    max_op = mybir.AluOpType.max

    nc = bacc.Bacc("TRN2", target_bir_lowering=False, debug=False)

    feats = nc.dram_tensor("features", [B, T * FEAT], f32, kind="ExternalInput")
    wm1_d = nc.dram_tensor("wm1", [128, 128], f32r, kind="ExternalInput")
    wm2h_d = nc.dram_tensor("wm2h", [128, 128], h2dt, kind="ExternalInput")
    wm2f_d = nc.dram_tensor("wm2f", [2 * FEAT, 128], fdt, kind="ExternalInput")
    w3_d = nc.dram_tensor("w3col", [128, 2], h2dt, kind="ExternalInput")
    ident_d = nc.dram_tensor("ident", [128, 128], f32, kind="ExternalInput")
    bias_h2_d = nc.dram_tensor("bias_h2", [128, 1], f32, kind="ExternalInput")
    bias_h1_d = nc.dram_tensor("bias_h1", [128, 1], f32, kind="ExternalInput")
    bias_h1f_d = nc.dram_tensor("bias_h1f", [128, 1], f32, kind="ExternalInput")
    bias_d_d = nc.dram_tensor("bias_d", [128, 1], f32, kind="ExternalInput")
    out_d = nc.dram_tensor("deltas", [B, T], f32, kind="ExternalOutput")

    # d-accumulator width: 16 windows x 15 steps; f32r tiny matmuls need a
    # 2-col write (trample-then-fix), bf16 writes 1 col.
    DOUTW = NW16 * TQ + (2 if not H2BF else 0)

    with tile.TileContext(nc) as tc:
        with (
            tc.tile_pool(name="constp", bufs=1) as constp,
            tc.tile_pool(name="iop", bufs=3) as iop,
            tc.tile_pool(name="statep", bufs=2) as statep,
            tc.tile_pool(name="pspool", bufs=IOBUFS, space="PSUM") as pspool,
            tc.tile_pool(name="doutp", bufs=4, space="PSUM") as doutp,
            tc.tile_pool(name="dramp", bufs=1, space="DRAM") as dramp,
        ):
            wm1 = constp.tile_from(wm1_d[:, :], name="wm1_sb")
            wm2h = constp.tile_from(wm2h_d[:, :], name="wm2h_sb")
            wm2f = constp.tile_from(wm2f_d[:, :], name="wm2f_sb")
            w3sb = constp.tile_from(w3_d[:, :], name="w3_sb")
            ident = constp.tile_from(ident_d[:, :], name="ident_sb")
            bias_h2 = constp.tile_from(bias_h2_d[:, :], name="bias_h2_sb")
            bias_h1 = constp.tile_from(bias_h1_d[:, :], name="bias_h1_sb")
            bias_h1f = constp.tile_from(bias_h1f_d[:, :], name="bias_h1f_sb")
            bias_d = constp.tile_from(bias_d_d[:, :], name="bias_d_sb")

            fstage = dramp.tile([T * FEAT, B], fdt, name="fstage")

            def relu_bias(engine_is_act, dst, src, bias_ap):
                if engine_is_act:
                    nc.scalar.activation(dst, src, Relu, bias=bias_ap)
                else:
                    nc.vector.tensor_scalar(dst, src, bias_ap, 0.0,
                                            add_op, max_op)

            KT = KT_ENV  # steps per batched fT window DMA
            assert T % KT == 0
            NW = SC // 512  # 512-path windows per superchunk (prepass units)

            def prepass_load(sc, w):
                """DMA 512 paths' raw features into SBUF; return j-slices."""
                base = sc * SC
                p0 = base + w * 512
                ft = iop.tile([128, 4 * T * FEAT], f32, tag="Ftile", bufs=8,
                              name="Ftile")
                src3 = feats[p0:p0 + 512, :].rearrange("(j l) c -> l j c", l=128)
                dst3 = ft.rearrange("l (j c) -> l j c", j=4)
                nc.sync.dma_start(dst3, src3)
                return ft

            def prepass_k(sc, w, ft, k):
                """Transpose rows [100k, 100k+100) of window w into fstage."""
                base = sc * SC
                p0 = base + w * 512
                ps_tr = pspool.tile([128, 512], f32, tag="io", name="ps_tr")
                for j in range(4):
                    nc.tensor.transpose(
                        ps_tr[0:100, 128 * j:128 * (j + 1)],
                        ft[:, T * FEAT * j + 100 * k:T * FEAT * j + 100 * (k + 1)],
                        ident,
                    )
                stg = iop.tile([128, 512], fdt, tag="stg", name="stg")
                if (w + k) % 2 == 0:
                    nc.scalar.copy(stg[0:100, :], ps_tr[0:100, :])
                else:
                    nc.vector.tensor_copy(stg[0:100, :], ps_tr[0:100, :])
                nc.sync.dma_start(
                    fstage[100 * k:100 * (k + 1), p0:p0 + 512], stg[0:100, :]
                )

            def prepass_span(sc, w0, w1):
                """Stage 512-path windows [w0, w1) of sc into fstage."""
                fts = {w: prepass_load(sc, w) for w in range(w0, w1)}
                for k in range(3):
                    for w in range(w0, w1):
                        prepass_k(sc, w, fts[w], k)

            def prepass_initial(sc):
                """First superchunk: all window DMAs up front, transposes
                k-major across ALL windows so the T-loop (which consumes
                fstage rows in step order) can start after ~1/3 of the work.
                """
                fts = {w: prepass_load(sc, w) for w in range(NW)}
                for k in range(3):
                    for w in range(NW):
                        prepass_k(sc, w, fts[w], k)

            class Lane:
                pass

            def load_fwin(st, w):
                """Load fT for steps [KT*w, KT*w + KT): rows 0-4 group A, 5-9 B."""
                t0 = KT * w
                fTbig = iop.tile([2 * FEAT, KT * G], fdt, tag="fTbig", bufs=6,
                                 name="fTbig")
                for half, col in ((0, st.colA), (1, st.colB)):
                    src = fstage[FEAT * t0:FEAT * (t0 + KT), col]
                    src3 = src.rearrange("(k c) n -> c k n", c=FEAT)
                    dst3 = fTbig[FEAT * half:FEAT * (half + 1), :].rearrange(
                        "c (k n) -> c k n", n=G)
                    nc.sync.dma_start(dst3, src3)
                st.fT[w] = fTbig
                st.fT.pop(w - 2, None)

            def lane_init(sc):
                st = Lane()
                st.sc = sc
                base = sc * SC
                st.colA = slice(base, base + G)
                st.colB = slice(base + G, base + SC)
                st.fT = {}
                load_fwin(st, 0)
                load_fwin(st, 1)  # prefetch one window ahead
                st.h1 = statep.tile([128, G], f32r, tag="h1", bufs=6, name="h1")
                for c in range(NCH):
                    cs = slice(CH * c, CH * (c + 1))
                    ps = pspool.tile([128, CH], f32, tag="io", name="m2ps")
                    nc.tensor.matmul(ps, wm2f, st.fT[0][:, cs], start=True, stop=True)
                    relu_bias(c % 2 == 0, st.h1[:, cs], ps, bias_h1f)
                # d accumulators: [128 paths, 16 windows x 15 steps], one
                # per group, reused across the 4 quarter-T drains
                # (WAR-protected).
                st.dout = [doutp.tile([128, DOUTW], f32, tag="dout", bufs=4,
                                      name="dout") for _ in range(2)]
                st.outsb = [None, None]
                return st

            def drain(st, q):
                """dout quarter q (steps 15q..15q+14) -> outsb, bias b3 fused."""
                for g in range(2):
                    if st.outsb[g] is None:
                        st.outsb[g] = iop.tile([128, NW16 * T], f32, tag="outsb",
                                               bufs=4, name="outsb")
                    dst = st.outsb[g].rearrange("p (w s) -> p w s", s=T)[
                        :, :, TQ * q:TQ * (q + 1)]
                    if H2BF:
                        src = st.dout[g].rearrange("p (w s) -> p w s", s=TQ)
                    else:
                        src = st.dout[g][:, 0:NW16 * TQ].rearrange(
                            "p (s w) -> p w s", w=NW16)
                    nc.scalar.add(dst, src, bias_d)

            def out_dma(st):
                base = st.sc * SC
                for g in range(2):
                    p0 = base + g * G
                    dst3 = out_d[p0:p0 + G, :].rearrange("(w p) t -> p w t", p=128)
                    src3 = st.outsb[g].rearrange("p (w t) -> p w t", t=T)
                    nc.sync.dma_start(dst3, src3)

            def tiny_d(st, t, h2, c):
                """Transposed-d matmuls for chunk c: weight = h2 slice, rhs =
                W3 col; d_t lands partition-major at dout[g][:, col]."""
                tt = t % TQ
                for w in range(4 * c, 4 * (c + 1)):
                    ws = slice(128 * w, 128 * (w + 1))
                    if H2BF:
                        col = w * TQ + tt
                        for g, hs in ((0, slice(0, 64)), (1, slice(64, 128))):
                            nc.tensor.matmul(
                                st.dout[g][:, col:col + 1], h2[hs, ws],
                                w3sb[hs, 0:1], start=True, stop=True,
                                skip_group_check=True)
                    else:
                        # 2-col write tramples w+1 / next-step slot (fixed by
                        # its later real write)
                        col = tt * NW16 + w
                        for g, hs in ((0, slice(0, 64)), (1, slice(64, 128))):
                            nc.tensor.matmul(
                                st.dout[g][:, col:col + 2], h2[hs, ws],
                                w3sb[hs, 0:2], start=True, stop=True,
                                skip_group_check=True)

            def lane_step(st, t):
                # drain the previous quarter's d accumulator first thing, so
                # this step's tiny-d writes (WAR on the same tile) wait as
                # briefly as possible.
                if t % TQ == 0 and t > 0:
                    drain(st, t // TQ - 1)
                # M1: h2 = relu(diag(W2,W2).T @ h1 + b2)
                h2 = statep.tile([128, G], h2dt, tag="h2", bufs=6, name="h2")
                for c in range(NCH):
                    cs = slice(CH * c, CH * (c + 1))
                    ps = pspool.tile([128, CH], f32, tag="io", name="m1ps")
                    nc.tensor.matmul(ps, wm1, st.h1[:, cs], start=True, stop=True)
                    relu_bias(c % 2 == 0, h2[:, cs], ps, bias_h2)
                if t < T - 1:
                    # M2: h1_{t+1} = relu(W13diag.T @ h2 + W1f.T @ fT_{t+1} + bias)
                    w1, i1 = divmod(t + 1, KT)
                    if i1 == 0 and w1 + 1 < T // KT:
                        load_fwin(st, w1 + 1)  # prefetch one window ahead
                    st.h1 = statep.tile([128, G], f32r, tag="h1", bufs=6, name="h1")
                    for c in range(NCH):
                        cs = slice(CH * c, CH * (c + 1))
                        fs = slice(i1 * G + CH * c, i1 * G + CH * (c + 1))
                        ps = pspool.tile([128, CH], f32, tag="io", name="m2ps")
                        # feature matmul first: it depends only on an old DMA,
                        # so the in-order PE streams it while the fresh
                        # relu-h2 semaphore is still propagating.
                        nc.tensor.matmul(ps, wm2f, st.fT[w1][:, fs], start=True,
                                         stop=False)
                        nc.tensor.matmul(ps, wm2h, h2[:, cs], start=False,
                                         stop=True)
                        relu_bias(c % 2 == 1, st.h1[:, cs], ps, bias_h1)
                        # tiny-d right after M2(c): its dep (relu-h2(c)) is
                        # already satisfied, so no PE head-of-line blocking.
                        tiny_d(st, t, h2, c)
                else:
                    for c in range(NCH):
                        tiny_d(st, t, h2, c)

            # The first quad's prepass runs up front; the next quad's is
            # interleaved into this quad's T-loops, one 512-path window at a
            # time. LANES superchunks run step-interleaved for pipeline depth.
            NQD = NSC // LANES
            for s in range(LANES):
                prepass_initial(s)
            for quad in range(NQD):
                lanes = [lane_init(LANES * quad + i) for i in range(LANES)]
                nxts = [LANES * (quad + 1) + i for i in range(LANES)] \
                    if quad + 1 < NQD else []
                nspan = len(nxts) * NW
                gap_r = max(1, (T - 8) // max(nspan, 1))
                for r in range(T + LANES - 1):
                    for i, st in enumerate(lanes):
                        ti = r - i
                        if 0 <= ti < T:
                            lane_step(st, ti)
                    if nxts and r % gap_r == 0 and r // gap_r < nspan:
                        j = r // gap_r
                        prepass_span(nxts[j // NW], j % NW, j % NW + 1)
                for st in lanes:
                    drain(st, NQ4 - 1)
                    out_dma(st)

    nc.compile()
    return nc


def _get_nc():
    if "nc" not in _BUILD_CACHE:
        _BUILD_CACHE["nc"] = _build_nc()
    return _BUILD_CACHE["nc"]


def _host_prep(W1, b1, W2, b2, W3, b3):
    f32 = np.float32
    W1 = np.asarray(W1, f32)
    b1 = np.asarray(b1, f32)
    W2 = np.asarray(W2, f32)
    b2 = np.asarray(b2, f32)
    W3 = np.asarray(W3, f32)
    b3 = np.asarray(b3, f32)
    W1f = W1[0:FEAT, :]                    # (5, 64)
    w1d = W1[FEAT, :]                      # (64,)
    W13 = np.outer(W3[:, 0], w1d)          # (64, 64)  h1pre += W13.T @ h2

    wm1 = np.zeros((128, 128), f32)
    wm1[0:64, 0:64] = W2
    wm1[64:128, 64:128] = W2

    wm2h = np.zeros((128, 128), f32)
    wm2h[0:64, 0:64] = W13
    wm2h[64:128, 64:128] = W13

    wm2f = np.zeros((2 * FEAT, 128), f32)
    wm2f[0:FEAT, 0:64] = W1f
    wm2f[FEAT:2 * FEAT, 64:128] = W1f

    w3col = np.zeros((128, 2), f32)
    w3col[0:64, 0] = W3[:, 0]
    w3col[64:128, 0] = W3[:, 0]

    bias_h2 = np.concatenate([b2, b2]).reshape(128, 1)
    h1b = b1 + b3[0] * w1d
    bias_h1 = np.concatenate([h1b, h1b]).reshape(128, 1)
    bias_h1f = np.concatenate([b1, b1]).reshape(128, 1)
    bias_d = np.full((128, 1), b3[0], f32)
    ident = np.eye(128, dtype=f32)

    if H2BF:
        import ml_dtypes
        wm2h = wm2h.astype(ml_dtypes.bfloat16)
        w3col = w3col.astype(ml_dtypes.bfloat16)
    if FBF:
        import ml_dtypes
        wm2f = wm2f.astype(ml_dtypes.bfloat16)

    return dict(wm1=wm1, wm2h=wm2h, wm2f=wm2f, w3col=w3col, ident=ident,
                bias_h2=bias_h2, bias_h1=bias_h1, bias_h1f=bias_h1f,
                bias_d=bias_d)


def _run(inputs, trace=False):
    from concourse.bass_utils import run_bass_kernel_spmd

    features = np.ascontiguousarray(np.asarray(inputs["features"], np.float32))
    shared = _host_prep(inputs["W1"], inputs["b1"], inputs["W2"], inputs["b2"],
                        inputs["W3"], inputs["b3"])
    nc = _get_nc()

    in_maps = []
    for i in range(NCORES):
        m = dict(shared)
        m["features"] = features[i * B:(i + 1) * B].reshape(B, T * FEAT).copy()
        in_maps.append(m)

    res = run_bass_kernel_spmd(nc, in_maps, core_ids=list(range(NCORES)),
                               trace=trace)
    out = np.concatenate([r["deltas"] for r in res.results], axis=0)
    return out, res


def kernel(**inputs):
    out, _ = _run(inputs, trace=False)
    return out


def kernel_traced(**inputs):
    return _run(inputs, trace=True)
